# revision 12
# baseline (speedup 1.0000x reference)
"""Trainium2 Bass kernel for nn_Aggregator (GNN message passing), 8 NeuronCores.

Pipeline (single NEFF, SPMD over 8 cores):
  Stage A : entity_agg = scatter-mean of entity_emb[tail] onto head
            (head-range sharded; dma_gather of tail rows; one-hot matmul
             segment-sum into PSUM window accumulators; host-computed 1/cnt)
  AllGather: bf16 item table (item_agg rows, padded to 256B rows with a
            ones column) replicated to all cores; overlapped with the
            non-item half of stage A
  Stage B : user_mean = segment-sum(val * item_agg[im_item]) by user
  Stage C : per-nnz score = ||item - user_mean||, ex = exp(score/T - CMAX/T)
            (user_mean expanded on-chip via transposed one-hot matmul --
             no gather)
  Stage D : user_agg = segment-sum(ex * item)/segment-sum(ex) by user
            (denominator via a ones column in the item table rows)

Sharding: core c owns item segs [6250c,6250c+6250), entity segs
[50000+6250c, ...), users [6250c, ...). No all-reduce needed; one 12.8MB
all-gather. All host-side work is index preprocessing only (sort/pad/int16
packing); every float op runs on device.
"""

import numpy as np
import ml_dtypes

BF16 = ml_dtypes.bfloat16

N_ENT = 100_000
N_ITEMS = 50_000
N_USERS = 50_000
D = 64
TEMP = 0.2
NCORES = 8
EPC = 6250            # items per core == other-entities per core == users per core
WIN = 128             # segments per window
A_NWIN = 49           # windows per space (6250 = 48*128 + 106)
LAST_WIN_ROWS = EPC - 128 * (A_NWIN - 1)   # 106
A_ST = 4              # entity sub-tables of 25000 rows (int16 index range)
B_ST = 2              # item sub-tables of 25000 rows
SWW = 4               # windows per super-window
GMAX = 8              # max chunks (128 idx each) per dma_gather (ring limit)
CMAX = 11.0           # softmax stability shift (any value works mathematically;
                      # chosen > max observed score to keep exp() in range)
PAD_HREL = 384.0      # pad sentinel: never matches iota 0..127 (exact in bf16)


def _sws(n_win):
    return [(k, min(SWW, n_win - k)) for k in range(0, n_win, SWW)]


def make_plan(K, n_win, n_st, sws_list):
    """Deterministic emission plan shared by host packing and graph build.

    Groups are ordered (super-window, subtable, window); gathers batch
    consecutive chunks of one subtable run into <=GMAX-chunk dma_gathers.
    Returns dict with:
      groups : per-sw list of (w, s, kchunks, chunk_off)
      gathers: per-sw list of (s, gn, idx_col_off, chunk_off)
      order  : flat group list in stream order
    """
    K = np.asarray(K)
    plan = {"groups": [], "gathers": [], "order": []}
    coff = 0
    icol = 0
    for (w0, nw) in sws_list:
        glist, gath = [], []
        for s in range(n_st):
            run0 = coff
            for wi in range(nw):
                w = w0 + wi
                glist.append((w, s, int(K[w, s]), coff))
                plan["order"].append((w, s, int(K[w, s])))
                coff += int(K[w, s])
            rem = coff - run0
            st = run0
            while rem > 0:
                gn = min(GMAX, rem)
                gath.append((s, gn, icol, st))
                icol += gn * 8
                st += gn
                rem -= gn
        plan["groups"].append(glist)
        plan["gathers"].append(gath)
    plan["n_chunks"] = coff
    plan["n_icols"] = icol
    return plan


# ----------------------------------------------------------------------------
# host-side index preprocessing
# ----------------------------------------------------------------------------

def _plan_phase(seg_local, st, payload_idx, extra, n_win, n_st, ncores, core_of):
    """Per-(window,subtable) grouping, padded to the max-over-cores chunk
    count so all cores share one SPMD graph."""
    win = seg_local // WIN
    slot = seg_local % WIN
    gid = (core_of * n_win + win) * n_st + st
    order = np.argsort(gid, kind="stable")
    counts = np.bincount(gid[order], minlength=ncores * n_win * n_st)
    counts = counts.reshape(ncores, n_win, n_st)
    K = np.ceil(counts.max(axis=0) / 128.0).astype(np.int64)
    K = np.maximum(K, 1)
    starts = np.zeros(ncores * n_win * n_st + 1, np.int64)
    np.cumsum(counts.reshape(-1), out=starts[1:])
    return K, order, starts


def _fill_streams(K, order, starts, payload_idx, slot, extra, plan, ncores,
                  n_win, n_st):
    idx_s = payload_idx[order]
    slot_s = slot[order]
    extra_s = extra[order] if extra is not None else None
    nch = plan["n_chunks"]
    out = []
    for c in range(ncores):
        idx_out = np.zeros((nch * 128,), np.int16)
        hrel_out = np.full((nch * 128,), PAD_HREL, np.float32)
        ex_out = np.zeros((nch * 128,), np.float32) if extra is not None else None
        pos = 0
        for (w, s, k) in plan["order"]:
            g = (c * n_win + w) * n_st + s
            a, b = starts[g], starts[g + 1]
            n = b - a
            assert n <= k * 128
            idx_out[pos:pos + n] = idx_s[a:b]
            hrel_out[pos:pos + n] = slot_s[a:b]
            if extra is not None:
                ex_out[pos:pos + n] = extra_s[a:b]
            pos += k * 128
        out.append((idx_out, hrel_out, ex_out))
    return out


def _pack_core(entry, plan, with_val, K, n_win, n_st):
    idx_out, hrel_out, val_out = entry
    nch = plan["n_chunks"]
    # idx bands: per-gather blocks in plan order
    cols = []
    for gath in plan["gathers"]:
        for (s, gn, icol, coff) in gath:
            lst = idx_out[coff * 128:(coff + gn) * 128]
            cols.append(lst.reshape(gn * 8, 16).T)
    band = np.concatenate(cols, axis=1)
    idx_band = np.tile(band, (8, 1)).astype(np.int16)
    hrel = hrel_out.reshape(nch, 128).T.copy().astype(BF16)
    packed = {"idx": idx_band, "hrel": hrel}
    if with_val:
        packed["val"] = val_out.reshape(nch, 128).T.copy().astype(BF16)
        # uflat: window-major layout (window: st0 chunks then st1 chunks)
        uflat = np.full((nch * 128,), PAD_HREL, np.float32)
        pos = 0
        coffs = {(w, s): co for gl in plan["groups"] for (w, s, k, co) in gl}
        for w in range(n_win):
            for s in range(n_st):
                k = int(K[w, s])
                co = coffs[(w, s)]
                uflat[pos:pos + k * 128] = hrel_out[co * 128:(co + k) * 128]
                pos += k * 128
        packed["uflat"] = uflat.reshape(1, -1).astype(BF16)
    return packed


def prep_host(entity_emb, head, tail, im_user, im_item, im_val):
    entity_emb = np.asarray(entity_emb, dtype=np.float32)
    head = np.asarray(head).astype(np.int64)
    tail = np.asarray(tail).astype(np.int64)
    im_user = np.asarray(im_user).astype(np.int64)
    im_item = np.asarray(im_item).astype(np.int64)
    im_val = np.asarray(im_val, dtype=np.float32)

    ent_tab = np.zeros((N_ENT, 2 * D), dtype=BF16)
    ent_tab[:, :D] = entity_emb.astype(BF16)
    ent_tab[:, D] = np.float32(1.0).astype(BF16)

    cnt = np.bincount(head, minlength=N_ENT).astype(np.float32)
    recip = 1.0 / np.maximum(cnt, 1.0)

    # stage A: two spaces (item heads 0..48, other heads 49..97)
    is_other = head >= N_ITEMS
    core_a = np.where(is_other, (head - N_ITEMS) // EPC, head // EPC)
    lseg = np.where(is_other, head - N_ITEMS - EPC * core_a, head - EPC * core_a)
    lseg = lseg + is_other * (A_NWIN * WIN)
    st_a = tail // 25_000
    idx_a = (tail - 25_000 * st_a).astype(np.int64)
    KA, order_a, starts_a = _plan_phase(lseg, st_a, idx_a, None, 2 * A_NWIN,
                                        A_ST, NCORES, core_a)
    plan_a = make_plan(KA, 2 * A_NWIN, A_ST, _sws(A_NWIN) +
                       [(A_NWIN + w0, nw) for (w0, nw) in _sws(A_NWIN)])
    streams_a = _fill_streams(KA, order_a, starts_a, idx_a, lseg % WIN, None,
                              plan_a, NCORES, 2 * A_NWIN, A_ST)

    # B/C/D: shard nnz by user
    core_b = im_user // EPC
    lu = im_user - EPC * core_b
    st_b = im_item // 25_000
    idx_b = (im_item - 25_000 * st_b).astype(np.int64)
    KB, order_b, starts_b = _plan_phase(lu, st_b, idx_b, im_val, A_NWIN, B_ST,
                                        NCORES, core_b)
    plan_b = make_plan(KB, A_NWIN, B_ST, _sws(A_NWIN))
    streams_b = _fill_streams(KB, order_b, starts_b, idx_b, lu % WIN, im_val,
                              plan_b, NCORES, A_NWIN, B_ST)

    recip_tiles = []
    for c in range(NCORES):
        rt = np.ones((128, 2 * A_NWIN), np.float32)
        for w in range(A_NWIN):
            n = WIN if w < A_NWIN - 1 else LAST_WIN_ROWS
            rt[:n, w] = recip[EPC * c + 128 * w: EPC * c + 128 * w + n]
            rt[:n, A_NWIN + w] = recip[N_ITEMS + EPC * c + 128 * w:
                                       N_ITEMS + EPC * c + 128 * w + n]
        recip_tiles.append(rt)

    iota_row = np.broadcast_to(np.arange(128, dtype=np.float32)[None, :],
                               (128, 128)).astype(BF16).copy()
    iota_col = np.arange(128, dtype=np.float32)[:, None].copy()

    in_maps = []
    for c in range(NCORES):
        pa = _pack_core(streams_a[c], plan_a, False, KA, 2 * A_NWIN, A_ST)
        pb = _pack_core(streams_b[c], plan_b, True, KB, A_NWIN, B_ST)
        in_maps.append({
            "ent_tab": ent_tab,
            "a_idx": pa["idx"], "a_hrel": pa["hrel"],
            "a_recip": recip_tiles[c],
            "b_idx": pb["idx"], "b_urel": pb["hrel"],
            "b_val": pb["val"], "b_uflat": pb["uflat"],
            "iota_row": iota_row, "iota_col": iota_col,
        })
    return KA, KB, in_maps


# ----------------------------------------------------------------------------
# device graph
# ----------------------------------------------------------------------------

def build_graph(KA, KB):
    import concourse.bacc as bacc
    import concourse.mybir as mybir
    import concourse.tile as tile
    from contextlib import ExitStack

    f32 = mybir.dt.float32
    bf = mybir.dt.bfloat16
    i16 = mybir.dt.int16

    KA = np.asarray(KA); KB = np.asarray(KB)
    A_SWS = _sws(A_NWIN) + [(A_NWIN + w0, nw) for (w0, nw) in _sws(A_NWIN)]
    B_SWS = _sws(A_NWIN)
    plan_a = make_plan(KA, 2 * A_NWIN, A_ST, A_SWS)
    plan_b = make_plan(KB, A_NWIN, B_ST, B_SWS)
    KBw = KB.sum(axis=1)
    # window-major chunk offsets for uflat/ptw (window stream layout)
    WOFF = np.zeros(A_NWIN + 1, np.int64)
    np.cumsum(KBw, out=WOFF[1:])
    CWIN = {}
    for w in range(A_NWIN):
        acc = 0
        for s in range(B_ST):
            CWIN[(w, s)] = acc
            acc += int(KB[w, s])

    nc = bacc.Bacc(None, target_bir_lowering=False, debug=True)
    ent_tab = nc.declare_dram_parameter("ent_tab", [N_ENT, 2 * D], bf, isOutput=False)
    a_idx = nc.declare_dram_parameter("a_idx", [128, plan_a["n_icols"]], i16, isOutput=False)
    a_hrel = nc.declare_dram_parameter("a_hrel", [128, plan_a["n_chunks"]], bf, isOutput=False)
    a_recip = nc.declare_dram_parameter("a_recip", [128, 2 * A_NWIN], f32, isOutput=False)
    b_idx = nc.declare_dram_parameter("b_idx", [128, plan_b["n_icols"]], i16, isOutput=False)
    b_urel = nc.declare_dram_parameter("b_urel", [128, plan_b["n_chunks"]], bf, isOutput=False)
    b_val = nc.declare_dram_parameter("b_val", [128, plan_b["n_chunks"]], bf, isOutput=False)
    b_uflat = nc.declare_dram_parameter("b_uflat", [1, plan_b["n_chunks"] * 128], bf, isOutput=False)
    iota_row_p = nc.declare_dram_parameter("iota_row", [128, 128], bf, isOutput=False)
    iota_col_p = nc.declare_dram_parameter("iota_col", [128, 1], f32, isOutput=False)
    ent_out = nc.declare_dram_parameter("ent_out", [2 * EPC, D], f32, isOutput=True)
    user_out = nc.declare_dram_parameter("user_out", [EPC, D], f32, isOutput=True)

    eq = mybir.AluOpType.is_equal
    mul = mybir.AluOpType.mult
    sub = mybir.AluOpType.subtract

    with tile.TileContext(nc) as tc:
        with ExitStack() as ctx:
            cpool = ctx.enter_context(tc.tile_pool(name="const", bufs=1))
            astr = ctx.enter_context(tc.tile_pool(name="astr", bufs=3))
            pbp = ctx.enter_context(tc.tile_pool(name="pb", bufs=3))
            flp = ctx.enter_context(tc.tile_pool(name="flush", bufs=4))
            istp = ctx.enter_context(tc.tile_pool(name="ist", bufs=4))
            bstr = ctx.enter_context(tc.tile_pool(name="bstr", bufs=3))
            itp = ctx.enter_context(tc.tile_pool(name="item", bufs=4))
            cwp = ctx.enter_context(tc.tile_pool(name="cw", bufs=3))
            ump = ctx.enter_context(tc.tile_pool(name="um", bufs=10))
            scp = ctx.enter_context(tc.tile_pool(name="sc", bufs=4))
            drp = ctx.enter_context(tc.tile_pool(name="dram", bufs=1, space="DRAM"))
            psA = ctx.enter_context(tc.tile_pool(name="psA", bufs=2, space="PSUM"))
            psB = ctx.enter_context(tc.tile_pool(name="psB", bufs=2, space="PSUM"))
            psD = ctx.enter_context(tc.tile_pool(name="psD", bufs=2, space="PSUM"))
            psU = ctx.enter_context(tc.tile_pool(name="psU", bufs=2, space="PSUM"))

            iota_row = cpool.tile([128, 128], bf)
            nc.sync.dma_start(out=iota_row[:], in_=iota_row_p[:])
            iota_col = cpool.tile([128, 1], f32)
            nc.sync.dma_start(out=iota_col[:], in_=iota_col_p[:])
            recip_t = cpool.tile([128, 2 * A_NWIN], f32)
            nc.sync.dma_start(out=recip_t[:], in_=a_recip[:])
            exp_bias = cpool.tile([128, 1], f32)
            nc.vector.memset(exp_bias[:], -CMAX / TEMP)

            item_local = drp.tile([EPC, 2 * D], bf)
            item_full = drp.tile([N_ITEMS, 2 * D], bf, addr_space="Shared")

            item_stage = []
            for i in range(4):
                t = istp.tile([128, 2 * D], bf, tag="istage", name=f"istage{i}")
                nc.vector.memset(t[:, D:D + 1], 1.0)
                nc.vector.memset(t[:, D + 1:], 0.0)
                item_stage.append(t)

            # ---------------- stage A ----------------
            def stage_a_sw(swi, w0, nw):
                glist = plan_a["groups"][swi]
                gath = plan_a["gathers"][swi]
                c0 = glist[0][3]
                c1 = glist[-1][3] + glist[-1][2]
                i0 = gath[0][2]
                i1 = gath[-1][2] + gath[-1][1] * 8
                idx_t = astr.tile([128, i1 - i0], i16, tag="aidx")
                nc.sync.dma_start(out=idx_t[:], in_=a_idx[:, i0:i1])
                hr_t = astr.tile([128, c1 - c0], bf, tag="ahrel")
                nc.sync.dma_start(out=hr_t[:], in_=a_hrel[:, c0:c1])
                it_a = itp.tile([128, c1 - c0, 2 * D], bf, tag="it")
                for (s, gn, icol, coff) in gath:
                    nc.gpsimd.dma_gather(
                        out_ap=it_a[:, coff - c0:coff - c0 + gn, :],
                        in_ap=ent_tab[25_000 * s:25_000 * (s + 1), :],
                        idxs_ap=idx_t[:16, icol - i0:icol - i0 + gn * 8],
                        num_idxs=gn * 128, num_idxs_reg=gn * 128,
                        elem_size=2 * D)
                acc = psA.tile([128, nw, D], f32, tag="acca")
                nc.vector.memset(acc[:], 0.0)
                for (w, s, k, coff) in glist:
                    cl = coff - c0
                    for b0 in range(0, k, 8):
                        nb = min(8, k - b0)
                        pt = pbp.tile([128, 8, 128], bf, tag="pa")
                        nc.vector.tensor_tensor(
                            out=pt[:, 0:nb, :],
                            in0=iota_row[:].unsqueeze(1).to_broadcast([128, nb, 128]),
                            in1=hr_t[:, cl + b0:cl + b0 + nb]
                                .unsqueeze(2).to_broadcast([128, nb, 128]),
                            op=eq)
                        for j in range(nb):
                            nc.tensor.matmul(
                                out=acc[:, w - w0, :], lhsT=pt[:, j, :],
                                rhs=it_a[:, cl + b0 + j, 0:D],
                                start=False, stop=True, skip_group_check=True)
                for wi in range(nw):
                    w = w0 + wi
                    space = 1 if w >= A_NWIN else 0
                    lw = w - space * A_NWIN
                    nrows = WIN if lw < A_NWIN - 1 else LAST_WIN_ROWS
                    row0 = space * EPC + 128 * lw
                    stg = flp.tile([128, D], f32, tag="afl")
                    nc.vector.tensor_scalar(
                        out=stg[:], in0=acc[:, wi, :],
                        scalar1=recip_t[:, w:w + 1], scalar2=None, op0=mul)
                    nc.sync.dma_start(out=ent_out[row0:row0 + nrows, :],
                                      in_=stg[0:nrows, :])
                    if space == 0:
                        ist = item_stage[w % 4]
                        nc.vector.tensor_scalar(
                            out=ist[:, 0:D], in0=acc[:, wi, :],
                            scalar1=recip_t[:, w:w + 1], scalar2=None, op0=mul)
                        nc.sync.dma_start(
                            out=item_local[:][128 * lw:128 * lw + nrows, :],
                            in_=ist[0:nrows, :])

            n_sp0 = len(_sws(A_NWIN))
            for swi, (w0, nw) in enumerate(A_SWS[:n_sp0]):
                stage_a_sw(swi, w0, nw)

            # all-gather items; overlaps with space-1 stage A below
            nc.gpsimd.collective_compute(
                "AllGather", mybir.AluOpType.bypass,
                ins=[item_local[:].opt()], outs=[item_full[:].opt()],
                replica_groups=[list(range(NCORES))])


            # ---------------- stages B, C, D ----------------
            def b_sw(swi, w0, nw):
                glist = plan_b["groups"][swi]
                gath = plan_b["gathers"][swi]
                c0 = glist[0][3]
                c1 = glist[-1][3] + glist[-1][2]
                i0 = gath[0][2]
                i1 = gath[-1][2] + gath[-1][1] * 8
                idx_t = bstr.tile([128, i1 - i0], i16, tag="bidx")
                nc.sync.dma_start(out=idx_t[:], in_=b_idx[:, i0:i1])
                ur_t = bstr.tile([128, c1 - c0], bf, tag="burel")
                nc.sync.dma_start(out=ur_t[:], in_=b_urel[:, c0:c1])
                va_t = bstr.tile([128, c1 - c0], bf, tag="bval")
                nc.sync.dma_start(out=va_t[:], in_=b_val[:, c0:c1])

                it_sw = itp.tile([128, c1 - c0, 2 * D], bf, tag="it")
                for (s, gn, icol, coff) in gath:
                    nc.gpsimd.dma_gather(
                        out_ap=it_sw[:, coff - c0:coff - c0 + gn, :],
                        in_ap=item_full[:][25_000 * s:25_000 * (s + 1), :],
                        idxs_ap=idx_t[:16, icol - i0:icol - i0 + gn * 8],
                        num_idxs=gn * 128, num_idxs_reg=gn * 128,
                        elem_size=2 * D)

                Bacc = psB.tile([128, nw, D], f32, tag="accb")
                nc.vector.memset(Bacc[:], 0.0)
                Dacc = psD.tile([128, nw, D + 1], f32, tag="accd")
                nc.vector.memset(Dacc[:], 0.0)

                # ---- B: user_mean matmuls ----
                for (w, s, k, coff) in glist:
                    cl = coff - c0
                    for b0 in range(0, k, 8):
                        nb = min(8, k - b0)
                        pt = pbp.tile([128, 8, 128], bf, tag="pb")
                        nc.vector.tensor_tensor(
                            out=pt[:, 0:nb, :],
                            in0=iota_row[:].unsqueeze(1).to_broadcast([128, nb, 128]),
                            in1=ur_t[:, cl + b0:cl + b0 + nb]
                                .unsqueeze(2).to_broadcast([128, nb, 128]),
                            op=eq)
                        nc.vector.tensor_tensor(
                            out=pt[:, 0:nb, :], in0=pt[:, 0:nb, :],
                            in1=va_t[:, cl + b0:cl + b0 + nb]
                                .unsqueeze(2).to_broadcast([128, nb, 128]),
                            op=mul)
                        for j in range(nb):
                            nc.tensor.matmul(
                                out=Bacc[:, w - w0, :], lhsT=pt[:, j, :],
                                rhs=it_sw[:, cl + b0 + j, 0:D],
                                start=False, stop=True, skip_group_check=True)
                um_tiles = {}
                for wi in range(nw):
                    um = ump.tile([128, D], bf, tag="um")
                    nc.vector.tensor_copy(out=um[:], in_=Bacc[:, wi, :])
                    um_tiles[w0 + wi] = um

                # ---- C: ssq for all windows, then batched ACT ----
                sc_tiles = {}
                for wi in range(nw):
                    w = w0 + wi
                    kw = int(KBw[w])
                    ew = kw * 128
                    ub = cwp.tile([128, ew], bf, tag="ub")
                    nc.sync.dma_start(
                        out=ub[:],
                        in_=b_uflat[0:1, 128 * int(WOFF[w]):128 * int(WOFF[w]) + ew]
                            .broadcast_to([128, ew]))
                    ptw = cwp.tile([128, ew], bf, tag="ptw")
                    nc.vector.tensor_scalar(
                        out=ptw[:], in0=ub[:], scalar1=iota_col[:, :1],
                        scalar2=None, op0=eq)
                    ssq = scp.tile([128, kw], f32, tag="ssq")
                    for s in range(B_ST):
                        k = int(KB[w, s])
                        cl = [co for (ww, ss, kk, co) in glist
                              if ww == w and ss == s][0] - c0
                        cwl = CWIN[(w, s)]
                        for b0 in range(0, k, 8):
                            nb = min(8, k - b0)
                            ue = psU.tile([128, 8, D], f32, tag="ue")
                            for j in range(nb):
                                nc.tensor.matmul(
                                    out=ue[:, j, :],
                                    lhsT=ptw[:, (cwl + b0 + j) * 128:
                                             (cwl + b0 + j + 1) * 128],
                                    rhs=um_tiles[w][:],
                                    start=True, stop=True, skip_group_check=True)
                            dt = scp.tile([128, 8, D], bf, tag="dt")
                            nc.vector.tensor_tensor(
                                out=dt[:, 0:nb, :],
                                in0=it_sw[:, cl + b0:cl + b0 + nb, 0:D],
                                in1=ue[:, 0:nb, :], op=sub)
                            nc.vector.tensor_tensor(
                                out=dt[:, 0:nb, :], in0=dt[:, 0:nb, :],
                                in1=dt[:, 0:nb, :], op=mul)
                            nc.vector.reduce_sum(
                                out=ssq[:, cwl + b0:cwl + b0 + nb],
                                in_=dt[:, 0:nb, :], axis=mybir.AxisListType.X)
                    sc_tiles[w] = ssq
                for wi in range(nw):
                    w = w0 + wi
                    kw = int(KBw[w])
                    score = scp.tile([128, kw], f32, tag="score")
                    nc.scalar.activation(
                        out=score[:], in_=sc_tiles[w][:],
                        func=mybir.ActivationFunctionType.Sqrt)
                    sc_tiles[w] = score
                ex_tiles = {}
                for wi in range(nw):
                    w = w0 + wi
                    kw = int(KBw[w])
                    exf = scp.tile([128, kw], f32, tag="exf")
                    nc.scalar.activation(
                        out=exf[:], in_=sc_tiles[w][:],
                        func=mybir.ActivationFunctionType.Exp,
                        scale=1.0 / TEMP, bias=exp_bias[:, :1])
                    exb = scp.tile([128, kw], bf, tag="exb")
                    nc.vector.tensor_copy(out=exb[:], in_=exf[:])
                    ex_tiles[w] = exb
                # ---- D ----
                for wi in range(nw):
                    w = w0 + wi
                    exb = ex_tiles[w]
                    for s in range(B_ST):
                        k = int(KB[w, s])
                        cl = [co for (ww, ss, kk, co) in glist
                              if ww == w and ss == s][0] - c0
                        cwl = CWIN[(w, s)]
                        for b0 in range(0, k, 8):
                            nb = min(8, k - b0)
                            pt = pbp.tile([128, 8, 128], bf, tag="pd")
                            nc.vector.tensor_tensor(
                                out=pt[:, 0:nb, :],
                                in0=iota_row[:].unsqueeze(1).to_broadcast([128, nb, 128]),
                                in1=ur_t[:, cl + b0:cl + b0 + nb]
                                    .unsqueeze(2).to_broadcast([128, nb, 128]),
                                op=eq)
                            nc.vector.tensor_tensor(
                                out=pt[:, 0:nb, :], in0=pt[:, 0:nb, :],
                                in1=exb[:, cwl + b0:cwl + b0 + nb]
                                    .unsqueeze(2).to_broadcast([128, nb, 128]),
                                op=mul)
                            for j in range(nb):
                                nc.tensor.matmul(
                                    out=Dacc[:, wi, :], lhsT=pt[:, j, :],
                                    rhs=it_sw[:, cl + b0 + j, 0:D + 1],
                                    start=False, stop=True, skip_group_check=True)
                    nrows = WIN if w < A_NWIN - 1 else LAST_WIN_ROWS
                    rec = flp.tile([128, 1], f32, tag="drec")
                    nc.vector.reciprocal(out=rec[:], in_=Dacc[:, wi, D:D + 1])
                    ost = flp.tile([128, D], f32, tag="dfl")
                    nc.vector.tensor_scalar(
                        out=ost[:], in0=Dacc[:, wi, 0:D],
                        scalar1=rec[:, :1], scalar2=None, op0=mul)
                    nc.sync.dma_start(out=user_out[128 * w:128 * w + nrows, :],
                                      in_=ost[0:nrows, :])
            a1 = list(enumerate(A_SWS))[n_sp0:]
            for k in range(max(len(a1), len(B_SWS))):
                if k < len(a1):
                    swi, (w0, nw) = a1[k]
                    stage_a_sw(swi, w0, nw)
                if k < len(B_SWS):
                    w0, nw = B_SWS[k]
                    b_sw(k, w0, nw)
    nc.compile()
    return nc


# ----------------------------------------------------------------------------
# entry point
# ----------------------------------------------------------------------------

_CACHE = {}


def kernel(entity_emb, head, tail, im_user, im_item, im_val):
    from concourse.bass_utils import run_bass_kernel_spmd

    KA, KB, in_maps = prep_host(entity_emb, head, tail, im_user, im_item, im_val)

    key = (KA.tobytes(), KB.tobytes())
    if key not in _CACHE:
        _CACHE[key] = build_graph(KA, KB)
    nc = _CACHE[key]

    res = run_bass_kernel_spmd(nc, in_maps, list(range(NCORES)))
    entity_agg = np.zeros((N_ENT, D), np.float32)
    user_agg = np.zeros((N_USERS, D), np.float32)
    for c in range(NCORES):
        r = res.results[c]
        entity_agg[EPC * c:EPC * (c + 1)] = r["ent_out"][:EPC]
        entity_agg[N_ITEMS + EPC * c:N_ITEMS + EPC * (c + 1)] = r["ent_out"][EPC:]
        user_agg[EPC * c:EPC * (c + 1)] = r["user_out"]
    return user_agg, entity_agg


# revision 13
# speedup vs baseline: 1.2015x; 1.2015x over previous
"""Trainium2 Bass kernel for nn_Aggregator (GNN message passing), 8 NeuronCores.

Pipeline (single NEFF, SPMD over 8 cores):
  Stage A : entity_agg = scatter-mean of entity_emb[tail] onto head
            (head-range sharded; dma_gather of tail rows; one-hot matmul
             segment-sum into PSUM window accumulators; host-computed 1/cnt)
  AllGather: bf16 item table (item_agg rows, padded to 256B rows with a
            ones column) replicated to all cores; overlapped with the
            non-item half of stage A
  Stage B : user_mean = segment-sum(val * item_agg[im_item]) by user
  Stage C : per-nnz score = ||item - user_mean||, ex = exp(score/T - CMAX/T)
            (user_mean expanded on-chip via transposed one-hot matmul --
             no gather)
  Stage D : user_agg = segment-sum(ex * item)/segment-sum(ex) by user
            (denominator via a ones column in the item table rows)

Sharding: core c owns item segs [6250c,6250c+6250), entity segs
[50000+6250c, ...), users [6250c, ...). No all-reduce needed; one 12.8MB
all-gather. All host-side work is index preprocessing only (sort/pad/int16
packing); every float op runs on device.
"""

import numpy as np
import ml_dtypes

BF16 = ml_dtypes.bfloat16

N_ENT = 100_000
N_ITEMS = 50_000
N_USERS = 50_000
D = 64
TEMP = 0.2
NCORES = 8
EPC = 6250            # items per core == other-entities per core == users per core
WIN = 128             # segments per window
A_NWIN = 49           # windows per space (6250 = 48*128 + 106)
LAST_WIN_ROWS = EPC - 128 * (A_NWIN - 1)   # 106
A_ST = 4              # entity sub-tables of 25000 rows (int16 index range)
B_ST = 2              # item sub-tables of 25000 rows
SWW = 4               # windows per super-window
GMAX = 8              # max chunks (128 idx each) per dma_gather (ring limit)
CMAX = 11.0           # softmax stability shift (any value works mathematically;
                      # chosen > max observed score to keep exp() in range)
PAD_HREL = 384.0      # pad sentinel: never matches iota 0..127 (exact in bf16)


def _sws(n_win):
    return [(k, min(SWW, n_win - k)) for k in range(0, n_win, SWW)]


def make_plan(K, n_win, n_st, sws_list):
    """Deterministic emission plan shared by host packing and graph build.

    Groups are ordered (super-window, subtable, window); gathers batch
    consecutive chunks of one subtable run into <=GMAX-chunk dma_gathers.
    Returns dict with:
      groups : per-sw list of (w, s, kchunks, chunk_off)
      gathers: per-sw list of (s, gn, idx_col_off, chunk_off)
      order  : flat group list in stream order
    """
    K = np.asarray(K)
    plan = {"groups": [], "gathers": [], "order": []}
    coff = 0
    icol = 0
    for (w0, nw) in sws_list:
        glist, gath = [], []
        for s in range(n_st):
            run0 = coff
            for wi in range(nw):
                w = w0 + wi
                glist.append((w, s, int(K[w, s]), coff))
                plan["order"].append((w, s, int(K[w, s])))
                coff += int(K[w, s])
            rem = coff - run0
            st = run0
            while rem > 0:
                gn = min(GMAX, rem)
                gath.append((s, gn, icol, st))
                icol += gn * 8
                st += gn
                rem -= gn
        plan["groups"].append(glist)
        plan["gathers"].append(gath)
    plan["n_chunks"] = coff
    plan["n_icols"] = icol
    return plan


# ----------------------------------------------------------------------------
# host-side index preprocessing
# ----------------------------------------------------------------------------

def _plan_phase(seg_local, st, payload_idx, extra, n_win, n_st, ncores, core_of):
    """Per-(window,subtable) grouping, padded to the max-over-cores chunk
    count so all cores share one SPMD graph."""
    win = seg_local // WIN
    slot = seg_local % WIN
    gid = (core_of * n_win + win) * n_st + st
    order = np.argsort(gid, kind="stable")
    counts = np.bincount(gid[order], minlength=ncores * n_win * n_st)
    counts = counts.reshape(ncores, n_win, n_st)
    K = np.ceil(counts.max(axis=0) / 128.0).astype(np.int64)
    K = np.maximum(K, 1)
    starts = np.zeros(ncores * n_win * n_st + 1, np.int64)
    np.cumsum(counts.reshape(-1), out=starts[1:])
    return K, order, starts


def _fill_streams(K, order, starts, payload_idx, slot, extra, plan, ncores,
                  n_win, n_st):
    idx_s = payload_idx[order]
    slot_s = slot[order]
    extra_s = extra[order] if extra is not None else None
    nch = plan["n_chunks"]
    out = []
    for c in range(ncores):
        idx_out = np.zeros((nch * 128,), np.int16)
        hrel_out = np.full((nch * 128,), PAD_HREL, np.float32)
        ex_out = np.zeros((nch * 128,), np.float32) if extra is not None else None
        pos = 0
        for (w, s, k) in plan["order"]:
            g = (c * n_win + w) * n_st + s
            a, b = starts[g], starts[g + 1]
            n = b - a
            assert n <= k * 128
            idx_out[pos:pos + n] = idx_s[a:b]
            hrel_out[pos:pos + n] = slot_s[a:b]
            if extra is not None:
                ex_out[pos:pos + n] = extra_s[a:b]
            pos += k * 128
        out.append((idx_out, hrel_out, ex_out))
    return out


def _pack_core(entry, plan, with_val, K, n_win, n_st):
    idx_out, hrel_out, val_out = entry
    nch = plan["n_chunks"]
    # idx bands: per-gather blocks in plan order
    cols = []
    for gath in plan["gathers"]:
        for (s, gn, icol, coff) in gath:
            lst = idx_out[coff * 128:(coff + gn) * 128]
            cols.append(lst.reshape(gn * 8, 16).T)
    band = np.concatenate(cols, axis=1)
    idx_band = np.tile(band, (8, 1)).astype(np.int16)
    hrel = hrel_out.reshape(nch, 128).T.copy().astype(BF16)
    packed = {"idx": idx_band, "hrel": hrel}
    if with_val:
        packed["val"] = val_out.reshape(nch, 128).T.copy().astype(BF16)
        # uflat: window-major layout (window: st0 chunks then st1 chunks)
        uflat = np.full((nch * 128,), PAD_HREL, np.float32)
        pos = 0
        coffs = {(w, s): co for gl in plan["groups"] for (w, s, k, co) in gl}
        for w in range(n_win):
            for s in range(n_st):
                k = int(K[w, s])
                co = coffs[(w, s)]
                uflat[pos:pos + k * 128] = hrel_out[co * 128:(co + k) * 128]
                pos += k * 128
        packed["uflat"] = uflat.reshape(1, -1).astype(BF16)
    return packed


def prep_host(entity_emb, head, tail, im_user, im_item, im_val):
    entity_emb = np.asarray(entity_emb, dtype=np.float32)
    head = np.asarray(head).astype(np.int64)
    tail = np.asarray(tail).astype(np.int64)
    im_user = np.asarray(im_user).astype(np.int64)
    im_item = np.asarray(im_item).astype(np.int64)
    im_val = np.asarray(im_val, dtype=np.float32)

    ent_tab = np.zeros((N_ENT, 2 * D), dtype=BF16)
    ent_tab[:, :D] = entity_emb.astype(BF16)
    ent_tab[:, D] = np.float32(1.0).astype(BF16)

    cnt = np.bincount(head, minlength=N_ENT).astype(np.float32)
    recip = 1.0 / np.maximum(cnt, 1.0)

    # stage A: two spaces (item heads 0..48, other heads 49..97)
    is_other = head >= N_ITEMS
    core_a = np.where(is_other, (head - N_ITEMS) // EPC, head // EPC)
    lseg = np.where(is_other, head - N_ITEMS - EPC * core_a, head - EPC * core_a)
    lseg = lseg + is_other * (A_NWIN * WIN)
    st_a = tail // 25_000
    idx_a = (tail - 25_000 * st_a).astype(np.int64)
    KA, order_a, starts_a = _plan_phase(lseg, st_a, idx_a, None, 2 * A_NWIN,
                                        A_ST, NCORES, core_a)
    plan_a = make_plan(KA, 2 * A_NWIN, A_ST, _sws(A_NWIN) +
                       [(A_NWIN + w0, nw) for (w0, nw) in _sws(A_NWIN)])
    streams_a = _fill_streams(KA, order_a, starts_a, idx_a, lseg % WIN, None,
                              plan_a, NCORES, 2 * A_NWIN, A_ST)

    # B/C/D: shard nnz by user
    core_b = im_user // EPC
    lu = im_user - EPC * core_b
    st_b = im_item // 25_000
    idx_b = (im_item - 25_000 * st_b).astype(np.int64)
    KB, order_b, starts_b = _plan_phase(lu, st_b, idx_b, im_val, A_NWIN, B_ST,
                                        NCORES, core_b)
    plan_b = make_plan(KB, A_NWIN, B_ST, _sws(A_NWIN))
    streams_b = _fill_streams(KB, order_b, starts_b, idx_b, lu % WIN, im_val,
                              plan_b, NCORES, A_NWIN, B_ST)

    recip_tiles = []
    for c in range(NCORES):
        rt = np.ones((128, 2 * A_NWIN), np.float32)
        for w in range(A_NWIN):
            n = WIN if w < A_NWIN - 1 else LAST_WIN_ROWS
            rt[:n, w] = recip[EPC * c + 128 * w: EPC * c + 128 * w + n]
            rt[:n, A_NWIN + w] = recip[N_ITEMS + EPC * c + 128 * w:
                                       N_ITEMS + EPC * c + 128 * w + n]
        recip_tiles.append(rt)

    iota_row = np.broadcast_to(np.arange(128, dtype=np.float32)[None, :],
                               (128, 128)).astype(BF16).copy()
    iota_col = np.arange(128, dtype=np.float32)[:, None].copy()

    in_maps = []
    for c in range(NCORES):
        pa = _pack_core(streams_a[c], plan_a, False, KA, 2 * A_NWIN, A_ST)
        pb = _pack_core(streams_b[c], plan_b, True, KB, A_NWIN, B_ST)
        in_maps.append({
            "ent_tab": ent_tab,
            "a_idx": pa["idx"], "a_hrel": pa["hrel"],
            "a_recip": recip_tiles[c],
            "b_idx": pb["idx"], "b_urel": pb["hrel"],
            "b_val": pb["val"], "b_uflat": pb["uflat"],
            "iota_row": iota_row, "iota_col": iota_col,
        })
    return KA, KB, in_maps


# ----------------------------------------------------------------------------
# device graph
# ----------------------------------------------------------------------------

def build_graph(KA, KB):
    import concourse.bacc as bacc
    import concourse.mybir as mybir
    import concourse.tile as tile
    from contextlib import ExitStack

    f32 = mybir.dt.float32
    bf = mybir.dt.bfloat16
    i16 = mybir.dt.int16

    KA = np.asarray(KA); KB = np.asarray(KB)
    A_SWS = _sws(A_NWIN) + [(A_NWIN + w0, nw) for (w0, nw) in _sws(A_NWIN)]
    B_SWS = _sws(A_NWIN)
    plan_a = make_plan(KA, 2 * A_NWIN, A_ST, A_SWS)
    plan_b = make_plan(KB, A_NWIN, B_ST, B_SWS)
    KBw = KB.sum(axis=1)
    # window-major chunk offsets for uflat/ptw (window stream layout)
    WOFF = np.zeros(A_NWIN + 1, np.int64)
    np.cumsum(KBw, out=WOFF[1:])
    CWIN = {}
    for w in range(A_NWIN):
        acc = 0
        for s in range(B_ST):
            CWIN[(w, s)] = acc
            acc += int(KB[w, s])

    nc = bacc.Bacc(None, target_bir_lowering=False, debug=True)
    ent_tab = nc.declare_dram_parameter("ent_tab", [N_ENT, 2 * D], bf, isOutput=False)
    a_idx = nc.declare_dram_parameter("a_idx", [128, plan_a["n_icols"]], i16, isOutput=False)
    a_hrel = nc.declare_dram_parameter("a_hrel", [128, plan_a["n_chunks"]], bf, isOutput=False)
    a_recip = nc.declare_dram_parameter("a_recip", [128, 2 * A_NWIN], f32, isOutput=False)
    b_idx = nc.declare_dram_parameter("b_idx", [128, plan_b["n_icols"]], i16, isOutput=False)
    b_urel = nc.declare_dram_parameter("b_urel", [128, plan_b["n_chunks"]], bf, isOutput=False)
    b_val = nc.declare_dram_parameter("b_val", [128, plan_b["n_chunks"]], bf, isOutput=False)
    b_uflat = nc.declare_dram_parameter("b_uflat", [1, plan_b["n_chunks"] * 128], bf, isOutput=False)
    iota_row_p = nc.declare_dram_parameter("iota_row", [128, 128], bf, isOutput=False)
    iota_col_p = nc.declare_dram_parameter("iota_col", [128, 1], f32, isOutput=False)
    ent_out = nc.declare_dram_parameter("ent_out", [2 * EPC, D], f32, isOutput=True)
    user_out = nc.declare_dram_parameter("user_out", [EPC, D], f32, isOutput=True)

    eq = mybir.AluOpType.is_equal
    mul = mybir.AluOpType.mult
    sub = mybir.AluOpType.subtract

    with tile.TileContext(nc) as tc:
        with ExitStack() as ctx:
            cpool = ctx.enter_context(tc.tile_pool(name="const", bufs=1))
            astr = ctx.enter_context(tc.tile_pool(name="astr", bufs=3))
            pbp = ctx.enter_context(tc.tile_pool(name="pb", bufs=3))
            flp = ctx.enter_context(tc.tile_pool(name="flush", bufs=4))
            istp = ctx.enter_context(tc.tile_pool(name="ist", bufs=4))
            bstr = ctx.enter_context(tc.tile_pool(name="bstr", bufs=3))
            itpA = ctx.enter_context(tc.tile_pool(name="itemA", bufs=2))
            itp = ctx.enter_context(tc.tile_pool(name="item", bufs=3))
            cwp = ctx.enter_context(tc.tile_pool(name="cw", bufs=2))
            ump = ctx.enter_context(tc.tile_pool(name="um", bufs=10))
            scp = ctx.enter_context(tc.tile_pool(name="sc", bufs=4))
            drp = ctx.enter_context(tc.tile_pool(name="dram", bufs=1, space="DRAM"))
            psA = ctx.enter_context(tc.tile_pool(name="psA", bufs=2, space="PSUM"))
            psB = ctx.enter_context(tc.tile_pool(name="psB", bufs=2, space="PSUM"))
            psD = ctx.enter_context(tc.tile_pool(name="psD", bufs=2, space="PSUM"))
            psU = ctx.enter_context(tc.tile_pool(name="psU", bufs=2, space="PSUM"))

            iota_row = cpool.tile([128, 128], bf)
            nc.sync.dma_start(out=iota_row[:], in_=iota_row_p[:])
            iota_col = cpool.tile([128, 1], f32)
            nc.sync.dma_start(out=iota_col[:], in_=iota_col_p[:])
            recip_t = cpool.tile([128, 2 * A_NWIN], f32)
            nc.sync.dma_start(out=recip_t[:], in_=a_recip[:])
            exp_bias = cpool.tile([128, 1], f32)
            nc.vector.memset(exp_bias[:], -CMAX / TEMP)

            item_local = drp.tile([EPC, 2 * D], bf)
            item_full = drp.tile([N_ITEMS, 2 * D], bf, addr_space="Shared")

            item_stage = []
            for i in range(4):
                t = istp.tile([128, 2 * D], bf, tag="istage", name=f"istage{i}")
                nc.vector.memset(t[:, D:D + 1], 1.0)
                nc.vector.memset(t[:, D + 1:], 0.0)
                item_stage.append(t)

            # ---------------- stage A ----------------
            def stage_a_sw(swi, w0, nw):
                glist = plan_a["groups"][swi]
                gath = plan_a["gathers"][swi]
                c0 = glist[0][3]
                c1 = glist[-1][3] + glist[-1][2]
                i0 = gath[0][2]
                i1 = gath[-1][2] + gath[-1][1] * 8
                idx_t = astr.tile([128, i1 - i0], i16, tag="aidx")
                nc.sync.dma_start(out=idx_t[:], in_=a_idx[:, i0:i1])
                hr_t = astr.tile([128, c1 - c0], bf, tag="ahrel")
                nc.sync.dma_start(out=hr_t[:], in_=a_hrel[:, c0:c1])
                it_a = itpA.tile([128, c1 - c0, 2 * D], bf, tag="ita")
                for (s, gn, icol, coff) in gath:
                    nc.gpsimd.dma_gather(
                        out_ap=it_a[:, coff - c0:coff - c0 + gn, :],
                        in_ap=ent_tab[25_000 * s:25_000 * (s + 1), :],
                        idxs_ap=idx_t[:16, icol - i0:icol - i0 + gn * 8],
                        num_idxs=gn * 128, num_idxs_reg=gn * 128,
                        elem_size=2 * D)
                acc = psA.tile([128, nw, D], f32, tag="acca")
                nc.vector.memset(acc[:], 0.0)
                for (w, s, k, coff) in glist:
                    cl = coff - c0
                    for b0 in range(0, k, 8):
                        nb = min(8, k - b0)
                        pt = pbp.tile([128, 8, 128], bf, tag="pa")
                        nc.vector.tensor_tensor(
                            out=pt[:, 0:nb, :],
                            in0=iota_row[:].unsqueeze(1).to_broadcast([128, nb, 128]),
                            in1=hr_t[:, cl + b0:cl + b0 + nb]
                                .unsqueeze(2).to_broadcast([128, nb, 128]),
                            op=eq)
                        for j in range(nb):
                            nc.tensor.matmul(
                                out=acc[:, w - w0, :], lhsT=pt[:, j, :],
                                rhs=it_a[:, cl + b0 + j, 0:D],
                                start=False, stop=True, skip_group_check=True)
                for wi in range(nw):
                    w = w0 + wi
                    space = 1 if w >= A_NWIN else 0
                    lw = w - space * A_NWIN
                    nrows = WIN if lw < A_NWIN - 1 else LAST_WIN_ROWS
                    row0 = space * EPC + 128 * lw
                    stg = flp.tile([128, D], f32, tag="afl")
                    nc.vector.tensor_scalar(
                        out=stg[:], in0=acc[:, wi, :],
                        scalar1=recip_t[:, w:w + 1], scalar2=None, op0=mul)
                    nc.sync.dma_start(out=ent_out[row0:row0 + nrows, :],
                                      in_=stg[0:nrows, :])
                    if space == 0:
                        ist = item_stage[w % 4]
                        nc.vector.tensor_scalar(
                            out=ist[:, 0:D], in0=acc[:, wi, :],
                            scalar1=recip_t[:, w:w + 1], scalar2=None, op0=mul)
                        nc.sync.dma_start(
                            out=item_local[:][128 * lw:128 * lw + nrows, :],
                            in_=ist[0:nrows, :])

            n_sp0 = len(_sws(A_NWIN))
            for swi, (w0, nw) in enumerate(A_SWS[:n_sp0]):
                stage_a_sw(swi, w0, nw)

            # all-gather items; overlaps with space-1 stage A below
            nc.gpsimd.collective_compute(
                "AllGather", mybir.AluOpType.bypass,
                ins=[item_local[:].opt()], outs=[item_full[:].opt()],
                replica_groups=[list(range(NCORES))])


            # ---------------- stages B, C, D ----------------
            def b_sw(swi, w0, nw):
                glist = plan_b["groups"][swi]
                gath = plan_b["gathers"][swi]
                c0 = glist[0][3]
                c1 = glist[-1][3] + glist[-1][2]
                i0 = gath[0][2]
                i1 = gath[-1][2] + gath[-1][1] * 8
                idx_t = bstr.tile([128, i1 - i0], i16, tag="bidx")
                nc.sync.dma_start(out=idx_t[:], in_=b_idx[:, i0:i1])
                ur_t = bstr.tile([128, c1 - c0], bf, tag="burel")
                nc.sync.dma_start(out=ur_t[:], in_=b_urel[:, c0:c1])
                va_t = bstr.tile([128, c1 - c0], bf, tag="bval")
                nc.sync.dma_start(out=va_t[:], in_=b_val[:, c0:c1])

                it_sw = itp.tile([128, c1 - c0, 2 * D], bf, tag="it")
                for (s, gn, icol, coff) in gath:
                    nc.gpsimd.dma_gather(
                        out_ap=it_sw[:, coff - c0:coff - c0 + gn, :],
                        in_ap=item_full[:][25_000 * s:25_000 * (s + 1), :],
                        idxs_ap=idx_t[:16, icol - i0:icol - i0 + gn * 8],
                        num_idxs=gn * 128, num_idxs_reg=gn * 128,
                        elem_size=2 * D)

                Bacc = psB.tile([128, nw, D], f32, tag="accb")
                nc.vector.memset(Bacc[:], 0.0)
                Dacc = psD.tile([128, nw, D + 1], f32, tag="accd")
                nc.vector.memset(Dacc[:], 0.0)

                # ---- B: user_mean matmuls ----
                for (w, s, k, coff) in glist:
                    cl = coff - c0
                    for b0 in range(0, k, 8):
                        nb = min(8, k - b0)
                        pt = pbp.tile([128, 8, 128], bf, tag="pb")
                        nc.vector.tensor_tensor(
                            out=pt[:, 0:nb, :],
                            in0=iota_row[:].unsqueeze(1).to_broadcast([128, nb, 128]),
                            in1=ur_t[:, cl + b0:cl + b0 + nb]
                                .unsqueeze(2).to_broadcast([128, nb, 128]),
                            op=eq)
                        nc.vector.tensor_tensor(
                            out=pt[:, 0:nb, :], in0=pt[:, 0:nb, :],
                            in1=va_t[:, cl + b0:cl + b0 + nb]
                                .unsqueeze(2).to_broadcast([128, nb, 128]),
                            op=mul)
                        for j in range(nb):
                            nc.tensor.matmul(
                                out=Bacc[:, w - w0, :], lhsT=pt[:, j, :],
                                rhs=it_sw[:, cl + b0 + j, 0:D],
                                start=False, stop=True, skip_group_check=True)
                um_tiles = {}
                for wi in range(nw):
                    um = ump.tile([128, D], bf, tag="um")
                    nc.vector.tensor_copy(out=um[:], in_=Bacc[:, wi, :])
                    um_tiles[w0 + wi] = um

                # ---- C: ssq for all windows, then batched ACT ----
                sc_tiles = {}
                for wi in range(nw):
                    w = w0 + wi
                    kw = int(KBw[w])
                    ew = kw * 128
                    ub = cwp.tile([128, ew], bf, tag="ub")
                    nc.sync.dma_start(
                        out=ub[:],
                        in_=b_uflat[0:1, 128 * int(WOFF[w]):128 * int(WOFF[w]) + ew]
                            .broadcast_to([128, ew]))
                    ptw = cwp.tile([128, ew], bf, tag="ptw")
                    nc.vector.tensor_scalar(
                        out=ptw[:], in0=ub[:], scalar1=iota_col[:, :1],
                        scalar2=None, op0=eq)
                    ssq = scp.tile([128, kw], f32, tag="ssq")
                    for s in range(B_ST):
                        k = int(KB[w, s])
                        cl = [co for (ww, ss, kk, co) in glist
                              if ww == w and ss == s][0] - c0
                        cwl = CWIN[(w, s)]
                        for b0 in range(0, k, 8):
                            nb = min(8, k - b0)
                            ue = psU.tile([128, 8, D], f32, tag="ue")
                            for j in range(nb):
                                nc.tensor.matmul(
                                    out=ue[:, j, :],
                                    lhsT=ptw[:, (cwl + b0 + j) * 128:
                                             (cwl + b0 + j + 1) * 128],
                                    rhs=um_tiles[w][:],
                                    start=True, stop=True, skip_group_check=True)
                            dt = scp.tile([128, 8, D], bf, tag="dt")
                            nc.vector.tensor_tensor(
                                out=dt[:, 0:nb, :],
                                in0=it_sw[:, cl + b0:cl + b0 + nb, 0:D],
                                in1=ue[:, 0:nb, :], op=sub)
                            nc.vector.tensor_tensor(
                                out=dt[:, 0:nb, :], in0=dt[:, 0:nb, :],
                                in1=dt[:, 0:nb, :], op=mul)
                            nc.vector.reduce_sum(
                                out=ssq[:, cwl + b0:cwl + b0 + nb],
                                in_=dt[:, 0:nb, :], axis=mybir.AxisListType.X)
                    sc_tiles[w] = ssq
                for wi in range(nw):
                    w = w0 + wi
                    kw = int(KBw[w])
                    score = scp.tile([128, kw], f32, tag="score")
                    nc.scalar.activation(
                        out=score[:], in_=sc_tiles[w][:],
                        func=mybir.ActivationFunctionType.Sqrt)
                    sc_tiles[w] = score
                ex_tiles = {}
                for wi in range(nw):
                    w = w0 + wi
                    kw = int(KBw[w])
                    exf = scp.tile([128, kw], f32, tag="exf")
                    nc.scalar.activation(
                        out=exf[:], in_=sc_tiles[w][:],
                        func=mybir.ActivationFunctionType.Exp,
                        scale=1.0 / TEMP, bias=exp_bias[:, :1])
                    exb = scp.tile([128, kw], bf, tag="exb")
                    nc.vector.tensor_copy(out=exb[:], in_=exf[:])
                    ex_tiles[w] = exb
                # ---- D ----
                for wi in range(nw):
                    w = w0 + wi
                    exb = ex_tiles[w]
                    for s in range(B_ST):
                        k = int(KB[w, s])
                        cl = [co for (ww, ss, kk, co) in glist
                              if ww == w and ss == s][0] - c0
                        cwl = CWIN[(w, s)]
                        for b0 in range(0, k, 8):
                            nb = min(8, k - b0)
                            pt = pbp.tile([128, 8, 128], bf, tag="pd")
                            nc.vector.tensor_tensor(
                                out=pt[:, 0:nb, :],
                                in0=iota_row[:].unsqueeze(1).to_broadcast([128, nb, 128]),
                                in1=ur_t[:, cl + b0:cl + b0 + nb]
                                    .unsqueeze(2).to_broadcast([128, nb, 128]),
                                op=eq)
                            nc.vector.tensor_tensor(
                                out=pt[:, 0:nb, :], in0=pt[:, 0:nb, :],
                                in1=exb[:, cwl + b0:cwl + b0 + nb]
                                    .unsqueeze(2).to_broadcast([128, nb, 128]),
                                op=mul)
                            for j in range(nb):
                                nc.tensor.matmul(
                                    out=Dacc[:, wi, :], lhsT=pt[:, j, :],
                                    rhs=it_sw[:, cl + b0 + j, 0:D + 1],
                                    start=False, stop=True, skip_group_check=True)
                    nrows = WIN if w < A_NWIN - 1 else LAST_WIN_ROWS
                    rec = flp.tile([128, 1], f32, tag="drec")
                    nc.vector.reciprocal(out=rec[:], in_=Dacc[:, wi, D:D + 1])
                    ost = flp.tile([128, D], f32, tag="dfl")
                    nc.vector.tensor_scalar(
                        out=ost[:], in0=Dacc[:, wi, 0:D],
                        scalar1=rec[:, :1], scalar2=None, op0=mul)
                    nc.sync.dma_start(out=user_out[128 * w:128 * w + nrows, :],
                                      in_=ost[0:nrows, :])
            a1 = list(enumerate(A_SWS))[n_sp0:]
            for k in range(max(len(a1), len(B_SWS))):
                if k < len(a1):
                    swi, (w0, nw) = a1[k]
                    stage_a_sw(swi, w0, nw)
                if k < len(B_SWS):
                    w0, nw = B_SWS[k]
                    b_sw(k, w0, nw)
    nc.compile()
    return nc


# ----------------------------------------------------------------------------
# entry point
# ----------------------------------------------------------------------------

_CACHE = {}


def kernel(entity_emb, head, tail, im_user, im_item, im_val):
    from concourse.bass_utils import run_bass_kernel_spmd

    KA, KB, in_maps = prep_host(entity_emb, head, tail, im_user, im_item, im_val)

    key = (KA.tobytes(), KB.tobytes())
    if key not in _CACHE:
        _CACHE[key] = build_graph(KA, KB)
    nc = _CACHE[key]

    res = run_bass_kernel_spmd(nc, in_maps, list(range(NCORES)))
    entity_agg = np.zeros((N_ENT, D), np.float32)
    user_agg = np.zeros((N_USERS, D), np.float32)
    for c in range(NCORES):
        r = res.results[c]
        entity_agg[EPC * c:EPC * (c + 1)] = r["ent_out"][:EPC]
        entity_agg[N_ITEMS + EPC * c:N_ITEMS + EPC * (c + 1)] = r["ent_out"][EPC:]
        user_agg[EPC * c:EPC * (c + 1)] = r["user_out"]
    return user_agg, entity_agg


# revision 14
# speedup vs baseline: 1.2019x; 1.0003x over previous
"""Trainium2 Bass kernel for nn_Aggregator (GNN message passing), 8 NeuronCores.

Pipeline (single NEFF, SPMD over 8 cores):
  Stage A : entity_agg = scatter-mean of entity_emb[tail] onto head
            (head-range sharded; dma_gather of tail rows; one-hot matmul
             segment-sum into PSUM window accumulators; host-computed 1/cnt)
  AllGather: bf16 item table (item_agg rows, padded to 256B rows with a
            ones column) replicated to all cores; overlapped with the
            non-item half of stage A
  Stage B : user_mean = segment-sum(val * item_agg[im_item]) by user
  Stage C : per-nnz score = ||item - user_mean||, ex = exp(score/T - CMAX/T)
            (user_mean expanded on-chip via transposed one-hot matmul --
             no gather)
  Stage D : user_agg = segment-sum(ex * item)/segment-sum(ex) by user
            (denominator via a ones column in the item table rows)

Sharding: core c owns item segs [6250c,6250c+6250), entity segs
[50000+6250c, ...), users [6250c, ...). No all-reduce needed; one 12.8MB
all-gather. All host-side work is index preprocessing only (sort/pad/int16
packing); every float op runs on device.
"""

import numpy as np
import ml_dtypes

BF16 = ml_dtypes.bfloat16

N_ENT = 100_000
N_ITEMS = 50_000
N_USERS = 50_000
D = 64
TEMP = 0.2
NCORES = 8
EPC = 6250            # items per core == other-entities per core == users per core
WIN = 128             # segments per window
A_NWIN = 49           # windows per space (6250 = 48*128 + 106)
LAST_WIN_ROWS = EPC - 128 * (A_NWIN - 1)   # 106
A_ST = 4              # entity sub-tables of 25000 rows (int16 index range)
B_ST = 2              # item sub-tables of 25000 rows
SWW = 4               # windows per super-window
GMAX = 8              # max chunks (128 idx each) per dma_gather (ring limit)
CMAX = 11.0           # softmax stability shift (any value works mathematically;
                      # chosen > max observed score to keep exp() in range)
PAD_HREL = 384.0      # pad sentinel: never matches iota 0..127 (exact in bf16)


def _sws(n_win):
    return [(k, min(SWW, n_win - k)) for k in range(0, n_win, SWW)]


def make_plan(K, n_win, n_st, sws_list):
    """Deterministic emission plan shared by host packing and graph build.

    Groups are ordered (super-window, subtable, window); gathers batch
    consecutive chunks of one subtable run into <=GMAX-chunk dma_gathers.
    Returns dict with:
      groups : per-sw list of (w, s, kchunks, chunk_off)
      gathers: per-sw list of (s, gn, idx_col_off, chunk_off)
      order  : flat group list in stream order
    """
    K = np.asarray(K)
    plan = {"groups": [], "gathers": [], "order": []}
    coff = 0
    icol = 0
    for (w0, nw) in sws_list:
        glist, gath = [], []
        for s in range(n_st):
            run0 = coff
            for wi in range(nw):
                w = w0 + wi
                glist.append((w, s, int(K[w, s]), coff))
                plan["order"].append((w, s, int(K[w, s])))
                coff += int(K[w, s])
            rem = coff - run0
            st = run0
            while rem > 0:
                gn = min(GMAX, rem)
                gath.append((s, gn, icol, st))
                icol += gn * 8
                st += gn
                rem -= gn
        plan["groups"].append(glist)
        plan["gathers"].append(gath)
    plan["n_chunks"] = coff
    plan["n_icols"] = icol
    return plan


# ----------------------------------------------------------------------------
# host-side index preprocessing
# ----------------------------------------------------------------------------

def _plan_phase(seg_local, st, payload_idx, extra, n_win, n_st, ncores, core_of):
    """Per-(window,subtable) grouping, padded to the max-over-cores chunk
    count so all cores share one SPMD graph."""
    win = seg_local // WIN
    slot = seg_local % WIN
    gid = (core_of * n_win + win) * n_st + st
    order = np.argsort(gid, kind="stable")
    counts = np.bincount(gid[order], minlength=ncores * n_win * n_st)
    counts = counts.reshape(ncores, n_win, n_st)
    K = np.ceil(counts.max(axis=0) / 128.0).astype(np.int64)
    K = np.maximum(K, 1)
    starts = np.zeros(ncores * n_win * n_st + 1, np.int64)
    np.cumsum(counts.reshape(-1), out=starts[1:])
    return K, order, starts


def _fill_streams(K, order, starts, payload_idx, slot, extra, plan, ncores,
                  n_win, n_st):
    idx_s = payload_idx[order]
    slot_s = slot[order]
    extra_s = extra[order] if extra is not None else None
    nch = plan["n_chunks"]
    out = []
    for c in range(ncores):
        idx_out = np.zeros((nch * 128,), np.int16)
        hrel_out = np.full((nch * 128,), PAD_HREL, np.float32)
        ex_out = np.zeros((nch * 128,), np.float32) if extra is not None else None
        pos = 0
        for (w, s, k) in plan["order"]:
            g = (c * n_win + w) * n_st + s
            a, b = starts[g], starts[g + 1]
            n = b - a
            assert n <= k * 128
            idx_out[pos:pos + n] = idx_s[a:b]
            hrel_out[pos:pos + n] = slot_s[a:b]
            if extra is not None:
                ex_out[pos:pos + n] = extra_s[a:b]
            pos += k * 128
        out.append((idx_out, hrel_out, ex_out))
    return out


def _pack_core(entry, plan, with_val, K, n_win, n_st):
    idx_out, hrel_out, val_out = entry
    nch = plan["n_chunks"]
    # idx bands: per-gather blocks in plan order
    cols = []
    for gath in plan["gathers"]:
        for (s, gn, icol, coff) in gath:
            lst = idx_out[coff * 128:(coff + gn) * 128]
            cols.append(lst.reshape(gn * 8, 16).T)
    band = np.concatenate(cols, axis=1)
    idx_band = np.tile(band, (8, 1)).astype(np.int16)
    hrel = hrel_out.reshape(nch, 128).T.copy().astype(BF16)
    packed = {"idx": idx_band, "hrel": hrel}
    if with_val:
        packed["val"] = val_out.reshape(nch, 128).T.copy().astype(BF16)
        # uflat: window-major layout (window: st0 chunks then st1 chunks)
        uflat = np.full((nch * 128,), PAD_HREL, np.float32)
        pos = 0
        coffs = {(w, s): co for gl in plan["groups"] for (w, s, k, co) in gl}
        for w in range(n_win):
            for s in range(n_st):
                k = int(K[w, s])
                co = coffs[(w, s)]
                uflat[pos:pos + k * 128] = hrel_out[co * 128:(co + k) * 128]
                pos += k * 128
        packed["uflat"] = uflat.reshape(1, -1).astype(BF16)
    return packed


def prep_host(entity_emb, head, tail, im_user, im_item, im_val):
    entity_emb = np.asarray(entity_emb, dtype=np.float32)
    head = np.asarray(head).astype(np.int64)
    tail = np.asarray(tail).astype(np.int64)
    im_user = np.asarray(im_user).astype(np.int64)
    im_item = np.asarray(im_item).astype(np.int64)
    im_val = np.asarray(im_val, dtype=np.float32)

    ent_tab = np.zeros((N_ENT, 2 * D), dtype=BF16)
    ent_tab[:, :D] = entity_emb.astype(BF16)
    ent_tab[:, D] = np.float32(1.0).astype(BF16)

    cnt = np.bincount(head, minlength=N_ENT).astype(np.float32)
    recip = 1.0 / np.maximum(cnt, 1.0)

    # stage A: two spaces (item heads 0..48, other heads 49..97)
    is_other = head >= N_ITEMS
    core_a = np.where(is_other, (head - N_ITEMS) // EPC, head // EPC)
    lseg = np.where(is_other, head - N_ITEMS - EPC * core_a, head - EPC * core_a)
    lseg = lseg + is_other * (A_NWIN * WIN)
    st_a = tail // 25_000
    idx_a = (tail - 25_000 * st_a).astype(np.int64)
    KA, order_a, starts_a = _plan_phase(lseg, st_a, idx_a, None, 2 * A_NWIN,
                                        A_ST, NCORES, core_a)
    plan_a = make_plan(KA, 2 * A_NWIN, A_ST, _sws(A_NWIN) +
                       [(A_NWIN + w0, nw) for (w0, nw) in _sws(A_NWIN)])
    streams_a = _fill_streams(KA, order_a, starts_a, idx_a, lseg % WIN, None,
                              plan_a, NCORES, 2 * A_NWIN, A_ST)

    # B/C/D: shard nnz by user
    core_b = im_user // EPC
    lu = im_user - EPC * core_b
    st_b = im_item // 25_000
    idx_b = (im_item - 25_000 * st_b).astype(np.int64)
    KB, order_b, starts_b = _plan_phase(lu, st_b, idx_b, im_val, A_NWIN, B_ST,
                                        NCORES, core_b)
    plan_b = make_plan(KB, A_NWIN, B_ST, _sws(A_NWIN))
    streams_b = _fill_streams(KB, order_b, starts_b, idx_b, lu % WIN, im_val,
                              plan_b, NCORES, A_NWIN, B_ST)

    recip_tiles = []
    for c in range(NCORES):
        rt = np.ones((128, 2 * A_NWIN), np.float32)
        for w in range(A_NWIN):
            n = WIN if w < A_NWIN - 1 else LAST_WIN_ROWS
            rt[:n, w] = recip[EPC * c + 128 * w: EPC * c + 128 * w + n]
            rt[:n, A_NWIN + w] = recip[N_ITEMS + EPC * c + 128 * w:
                                       N_ITEMS + EPC * c + 128 * w + n]
        recip_tiles.append(rt)

    iota_row = np.broadcast_to(np.arange(128, dtype=np.float32)[None, :],
                               (128, 128)).astype(BF16).copy()
    iota_col = np.arange(128, dtype=np.float32)[:, None].copy()

    in_maps = []
    for c in range(NCORES):
        pa = _pack_core(streams_a[c], plan_a, False, KA, 2 * A_NWIN, A_ST)
        pb = _pack_core(streams_b[c], plan_b, True, KB, A_NWIN, B_ST)
        in_maps.append({
            "ent_tab": ent_tab,
            "a_idx": pa["idx"], "a_hrel": pa["hrel"],
            "a_recip": recip_tiles[c],
            "b_idx": pb["idx"], "b_urel": pb["hrel"],
            "b_val": pb["val"], "b_uflat": pb["uflat"],
            "iota_row": iota_row, "iota_col": iota_col,
        })
    return KA, KB, in_maps


# ----------------------------------------------------------------------------
# device graph
# ----------------------------------------------------------------------------

def build_graph(KA, KB):
    import concourse.bacc as bacc
    import concourse.mybir as mybir
    import concourse.tile as tile
    from contextlib import ExitStack

    f32 = mybir.dt.float32
    bf = mybir.dt.bfloat16
    i16 = mybir.dt.int16

    KA = np.asarray(KA); KB = np.asarray(KB)
    A_SWS = _sws(A_NWIN) + [(A_NWIN + w0, nw) for (w0, nw) in _sws(A_NWIN)]
    B_SWS = _sws(A_NWIN)
    plan_a = make_plan(KA, 2 * A_NWIN, A_ST, A_SWS)
    plan_b = make_plan(KB, A_NWIN, B_ST, B_SWS)
    KBw = KB.sum(axis=1)
    # window-major chunk offsets for uflat/ptw (window stream layout)
    WOFF = np.zeros(A_NWIN + 1, np.int64)
    np.cumsum(KBw, out=WOFF[1:])
    CWIN = {}
    for w in range(A_NWIN):
        acc = 0
        for s in range(B_ST):
            CWIN[(w, s)] = acc
            acc += int(KB[w, s])

    nc = bacc.Bacc(None, target_bir_lowering=False, debug=True)
    ent_tab = nc.declare_dram_parameter("ent_tab", [N_ENT, 2 * D], bf, isOutput=False)
    a_idx = nc.declare_dram_parameter("a_idx", [128, plan_a["n_icols"]], i16, isOutput=False)
    a_hrel = nc.declare_dram_parameter("a_hrel", [128, plan_a["n_chunks"]], bf, isOutput=False)
    a_recip = nc.declare_dram_parameter("a_recip", [128, 2 * A_NWIN], f32, isOutput=False)
    b_idx = nc.declare_dram_parameter("b_idx", [128, plan_b["n_icols"]], i16, isOutput=False)
    b_urel = nc.declare_dram_parameter("b_urel", [128, plan_b["n_chunks"]], bf, isOutput=False)
    b_val = nc.declare_dram_parameter("b_val", [128, plan_b["n_chunks"]], bf, isOutput=False)
    b_uflat = nc.declare_dram_parameter("b_uflat", [1, plan_b["n_chunks"] * 128], bf, isOutput=False)
    iota_row_p = nc.declare_dram_parameter("iota_row", [128, 128], bf, isOutput=False)
    iota_col_p = nc.declare_dram_parameter("iota_col", [128, 1], f32, isOutput=False)
    ent_out = nc.declare_dram_parameter("ent_out", [2 * EPC, D], f32, isOutput=True)
    user_out = nc.declare_dram_parameter("user_out", [EPC, D], f32, isOutput=True)

    eq = mybir.AluOpType.is_equal
    mul = mybir.AluOpType.mult
    sub = mybir.AluOpType.subtract

    with tile.TileContext(nc) as tc:
        with ExitStack() as ctx:
            cpool = ctx.enter_context(tc.tile_pool(name="const", bufs=1))
            astr = ctx.enter_context(tc.tile_pool(name="astr", bufs=3))
            pbp = ctx.enter_context(tc.tile_pool(name="pb", bufs=3))
            flp = ctx.enter_context(tc.tile_pool(name="flush", bufs=4))
            istp = ctx.enter_context(tc.tile_pool(name="ist", bufs=4))
            bstr = ctx.enter_context(tc.tile_pool(name="bstr", bufs=3))
            itpA = ctx.enter_context(tc.tile_pool(name="itemA", bufs=2))
            itp = ctx.enter_context(tc.tile_pool(name="item", bufs=4))
            cwp = ctx.enter_context(tc.tile_pool(name="cw", bufs=2))
            ump = ctx.enter_context(tc.tile_pool(name="um", bufs=10))
            scp = ctx.enter_context(tc.tile_pool(name="sc", bufs=4))
            drp = ctx.enter_context(tc.tile_pool(name="dram", bufs=1, space="DRAM"))
            psA = ctx.enter_context(tc.tile_pool(name="psA", bufs=2, space="PSUM"))
            psB = ctx.enter_context(tc.tile_pool(name="psB", bufs=2, space="PSUM"))
            psD = ctx.enter_context(tc.tile_pool(name="psD", bufs=2, space="PSUM"))
            psU = ctx.enter_context(tc.tile_pool(name="psU", bufs=2, space="PSUM"))

            iota_row = cpool.tile([128, 128], bf)
            nc.sync.dma_start(out=iota_row[:], in_=iota_row_p[:])
            iota_col = cpool.tile([128, 1], f32)
            nc.sync.dma_start(out=iota_col[:], in_=iota_col_p[:])
            recip_t = cpool.tile([128, 2 * A_NWIN], f32)
            nc.sync.dma_start(out=recip_t[:], in_=a_recip[:])
            exp_bias = cpool.tile([128, 1], f32)
            nc.vector.memset(exp_bias[:], -CMAX / TEMP)

            item_local = drp.tile([EPC, 2 * D], bf)
            item_full = drp.tile([N_ITEMS, 2 * D], bf, addr_space="Shared")

            item_stage = []
            for i in range(4):
                t = istp.tile([128, 2 * D], bf, tag="istage", name=f"istage{i}")
                nc.vector.memset(t[:, D:D + 1], 1.0)
                nc.vector.memset(t[:, D + 1:], 0.0)
                item_stage.append(t)

            # ---------------- stage A ----------------
            def stage_a_sw(swi, w0, nw):
                glist = plan_a["groups"][swi]
                gath = plan_a["gathers"][swi]
                c0 = glist[0][3]
                c1 = glist[-1][3] + glist[-1][2]
                i0 = gath[0][2]
                i1 = gath[-1][2] + gath[-1][1] * 8
                idx_t = astr.tile([128, i1 - i0], i16, tag="aidx")
                nc.sync.dma_start(out=idx_t[:], in_=a_idx[:, i0:i1])
                hr_t = astr.tile([128, c1 - c0], bf, tag="ahrel")
                nc.sync.dma_start(out=hr_t[:], in_=a_hrel[:, c0:c1])
                it_a = itpA.tile([128, c1 - c0, 2 * D], bf, tag="ita")
                for (s, gn, icol, coff) in gath:
                    nc.gpsimd.dma_gather(
                        out_ap=it_a[:, coff - c0:coff - c0 + gn, :],
                        in_ap=ent_tab[25_000 * s:25_000 * (s + 1), :],
                        idxs_ap=idx_t[:16, icol - i0:icol - i0 + gn * 8],
                        num_idxs=gn * 128, num_idxs_reg=gn * 128,
                        elem_size=2 * D)
                acc = psA.tile([128, nw, D], f32, tag="acca")
                nc.vector.memset(acc[:], 0.0)
                for (w, s, k, coff) in glist:
                    cl = coff - c0
                    for b0 in range(0, k, 8):
                        nb = min(8, k - b0)
                        pt = pbp.tile([128, 8, 128], bf, tag="pa")
                        nc.vector.tensor_tensor(
                            out=pt[:, 0:nb, :],
                            in0=iota_row[:].unsqueeze(1).to_broadcast([128, nb, 128]),
                            in1=hr_t[:, cl + b0:cl + b0 + nb]
                                .unsqueeze(2).to_broadcast([128, nb, 128]),
                            op=eq)
                        for j in range(nb):
                            nc.tensor.matmul(
                                out=acc[:, w - w0, :], lhsT=pt[:, j, :],
                                rhs=it_a[:, cl + b0 + j, 0:D],
                                start=False, stop=True, skip_group_check=True)
                for wi in range(nw):
                    w = w0 + wi
                    space = 1 if w >= A_NWIN else 0
                    lw = w - space * A_NWIN
                    nrows = WIN if lw < A_NWIN - 1 else LAST_WIN_ROWS
                    row0 = space * EPC + 128 * lw
                    stg = flp.tile([128, D], f32, tag="afl")
                    nc.vector.tensor_scalar(
                        out=stg[:], in0=acc[:, wi, :],
                        scalar1=recip_t[:, w:w + 1], scalar2=None, op0=mul)
                    nc.sync.dma_start(out=ent_out[row0:row0 + nrows, :],
                                      in_=stg[0:nrows, :])
                    if space == 0:
                        ist = item_stage[w % 4]
                        nc.vector.tensor_scalar(
                            out=ist[:, 0:D], in0=acc[:, wi, :],
                            scalar1=recip_t[:, w:w + 1], scalar2=None, op0=mul)
                        nc.sync.dma_start(
                            out=item_local[:][128 * lw:128 * lw + nrows, :],
                            in_=ist[0:nrows, :])

            n_sp0 = len(_sws(A_NWIN))
            for swi, (w0, nw) in enumerate(A_SWS[:n_sp0]):
                stage_a_sw(swi, w0, nw)

            # all-gather items; overlaps with space-1 stage A below
            nc.gpsimd.collective_compute(
                "AllGather", mybir.AluOpType.bypass,
                ins=[item_local[:].opt()], outs=[item_full[:].opt()],
                replica_groups=[list(range(NCORES))])


            # ---------------- stages B, C, D ----------------
            def b_sw(swi, w0, nw):
                glist = plan_b["groups"][swi]
                gath = plan_b["gathers"][swi]
                c0 = glist[0][3]
                c1 = glist[-1][3] + glist[-1][2]
                i0 = gath[0][2]
                i1 = gath[-1][2] + gath[-1][1] * 8
                idx_t = bstr.tile([128, i1 - i0], i16, tag="bidx")
                nc.sync.dma_start(out=idx_t[:], in_=b_idx[:, i0:i1])
                ur_t = bstr.tile([128, c1 - c0], bf, tag="burel")
                nc.sync.dma_start(out=ur_t[:], in_=b_urel[:, c0:c1])
                va_t = bstr.tile([128, c1 - c0], bf, tag="bval")
                nc.sync.dma_start(out=va_t[:], in_=b_val[:, c0:c1])

                it_sw = itp.tile([128, c1 - c0, 2 * D], bf, tag="it")
                for (s, gn, icol, coff) in gath:
                    nc.gpsimd.dma_gather(
                        out_ap=it_sw[:, coff - c0:coff - c0 + gn, :],
                        in_ap=item_full[:][25_000 * s:25_000 * (s + 1), :],
                        idxs_ap=idx_t[:16, icol - i0:icol - i0 + gn * 8],
                        num_idxs=gn * 128, num_idxs_reg=gn * 128,
                        elem_size=2 * D)

                Bacc = psB.tile([128, nw, D], f32, tag="accb")
                nc.vector.memset(Bacc[:], 0.0)
                Dacc = psD.tile([128, nw, D + 1], f32, tag="accd")
                nc.vector.memset(Dacc[:], 0.0)

                # ---- B: user_mean matmuls ----
                for (w, s, k, coff) in glist:
                    cl = coff - c0
                    for b0 in range(0, k, 8):
                        nb = min(8, k - b0)
                        pt = pbp.tile([128, 8, 128], bf, tag="pb")
                        nc.vector.tensor_tensor(
                            out=pt[:, 0:nb, :],
                            in0=iota_row[:].unsqueeze(1).to_broadcast([128, nb, 128]),
                            in1=ur_t[:, cl + b0:cl + b0 + nb]
                                .unsqueeze(2).to_broadcast([128, nb, 128]),
                            op=eq)
                        nc.vector.tensor_tensor(
                            out=pt[:, 0:nb, :], in0=pt[:, 0:nb, :],
                            in1=va_t[:, cl + b0:cl + b0 + nb]
                                .unsqueeze(2).to_broadcast([128, nb, 128]),
                            op=mul)
                        for j in range(nb):
                            nc.tensor.matmul(
                                out=Bacc[:, w - w0, :], lhsT=pt[:, j, :],
                                rhs=it_sw[:, cl + b0 + j, 0:D],
                                start=False, stop=True, skip_group_check=True)
                um_tiles = {}
                for wi in range(nw):
                    um = ump.tile([128, D], bf, tag="um")
                    nc.vector.tensor_copy(out=um[:], in_=Bacc[:, wi, :])
                    um_tiles[w0 + wi] = um

                # ---- C: ssq for all windows, then batched ACT ----
                sc_tiles = {}
                for wi in range(nw):
                    w = w0 + wi
                    kw = int(KBw[w])
                    ew = kw * 128
                    ub = cwp.tile([128, ew], bf, tag="ub")
                    nc.sync.dma_start(
                        out=ub[:],
                        in_=b_uflat[0:1, 128 * int(WOFF[w]):128 * int(WOFF[w]) + ew]
                            .broadcast_to([128, ew]))
                    ptw = cwp.tile([128, ew], bf, tag="ptw")
                    nc.vector.tensor_scalar(
                        out=ptw[:], in0=ub[:], scalar1=iota_col[:, :1],
                        scalar2=None, op0=eq)
                    ssq = scp.tile([128, kw], f32, tag="ssq")
                    for s in range(B_ST):
                        k = int(KB[w, s])
                        cl = [co for (ww, ss, kk, co) in glist
                              if ww == w and ss == s][0] - c0
                        cwl = CWIN[(w, s)]
                        for b0 in range(0, k, 8):
                            nb = min(8, k - b0)
                            ue = psU.tile([128, 8, D], f32, tag="ue")
                            for j in range(nb):
                                nc.tensor.matmul(
                                    out=ue[:, j, :],
                                    lhsT=ptw[:, (cwl + b0 + j) * 128:
                                             (cwl + b0 + j + 1) * 128],
                                    rhs=um_tiles[w][:],
                                    start=True, stop=True, skip_group_check=True)
                            dt = scp.tile([128, 8, D], bf, tag="dt")
                            nc.vector.tensor_tensor(
                                out=dt[:, 0:nb, :],
                                in0=it_sw[:, cl + b0:cl + b0 + nb, 0:D],
                                in1=ue[:, 0:nb, :], op=sub)
                            nc.vector.tensor_tensor(
                                out=dt[:, 0:nb, :], in0=dt[:, 0:nb, :],
                                in1=dt[:, 0:nb, :], op=mul)
                            nc.vector.reduce_sum(
                                out=ssq[:, cwl + b0:cwl + b0 + nb],
                                in_=dt[:, 0:nb, :], axis=mybir.AxisListType.X)
                    sc_tiles[w] = ssq
                for wi in range(nw):
                    w = w0 + wi
                    kw = int(KBw[w])
                    score = scp.tile([128, kw], f32, tag="score")
                    nc.scalar.activation(
                        out=score[:], in_=sc_tiles[w][:],
                        func=mybir.ActivationFunctionType.Sqrt)
                    sc_tiles[w] = score
                ex_tiles = {}
                for wi in range(nw):
                    w = w0 + wi
                    kw = int(KBw[w])
                    exf = scp.tile([128, kw], f32, tag="exf")
                    nc.scalar.activation(
                        out=exf[:], in_=sc_tiles[w][:],
                        func=mybir.ActivationFunctionType.Exp,
                        scale=1.0 / TEMP, bias=exp_bias[:, :1])
                    exb = scp.tile([128, kw], bf, tag="exb")
                    nc.vector.tensor_copy(out=exb[:], in_=exf[:])
                    ex_tiles[w] = exb
                # ---- D ----
                for wi in range(nw):
                    w = w0 + wi
                    exb = ex_tiles[w]
                    for s in range(B_ST):
                        k = int(KB[w, s])
                        cl = [co for (ww, ss, kk, co) in glist
                              if ww == w and ss == s][0] - c0
                        cwl = CWIN[(w, s)]
                        for b0 in range(0, k, 8):
                            nb = min(8, k - b0)
                            pt = pbp.tile([128, 8, 128], bf, tag="pd")
                            nc.vector.tensor_tensor(
                                out=pt[:, 0:nb, :],
                                in0=iota_row[:].unsqueeze(1).to_broadcast([128, nb, 128]),
                                in1=ur_t[:, cl + b0:cl + b0 + nb]
                                    .unsqueeze(2).to_broadcast([128, nb, 128]),
                                op=eq)
                            nc.vector.tensor_tensor(
                                out=pt[:, 0:nb, :], in0=pt[:, 0:nb, :],
                                in1=exb[:, cwl + b0:cwl + b0 + nb]
                                    .unsqueeze(2).to_broadcast([128, nb, 128]),
                                op=mul)
                            for j in range(nb):
                                nc.tensor.matmul(
                                    out=Dacc[:, wi, :], lhsT=pt[:, j, :],
                                    rhs=it_sw[:, cl + b0 + j, 0:D + 1],
                                    start=False, stop=True, skip_group_check=True)
                    nrows = WIN if w < A_NWIN - 1 else LAST_WIN_ROWS
                    rec = flp.tile([128, 1], f32, tag="drec")
                    nc.vector.reciprocal(out=rec[:], in_=Dacc[:, wi, D:D + 1])
                    ost = flp.tile([128, D], f32, tag="dfl")
                    nc.vector.tensor_scalar(
                        out=ost[:], in0=Dacc[:, wi, 0:D],
                        scalar1=rec[:, :1], scalar2=None, op0=mul)
                    nc.sync.dma_start(out=user_out[128 * w:128 * w + nrows, :],
                                      in_=ost[0:nrows, :])
            a1 = list(enumerate(A_SWS))[n_sp0:]
            for k in range(max(len(a1), len(B_SWS))):
                if k < len(a1):
                    swi, (w0, nw) = a1[k]
                    stage_a_sw(swi, w0, nw)
                if k < len(B_SWS):
                    w0, nw = B_SWS[k]
                    b_sw(k, w0, nw)
    nc.compile()
    return nc


# ----------------------------------------------------------------------------
# entry point
# ----------------------------------------------------------------------------

_CACHE = {}


def kernel(entity_emb, head, tail, im_user, im_item, im_val):
    from concourse.bass_utils import run_bass_kernel_spmd

    KA, KB, in_maps = prep_host(entity_emb, head, tail, im_user, im_item, im_val)

    key = (KA.tobytes(), KB.tobytes())
    if key not in _CACHE:
        _CACHE[key] = build_graph(KA, KB)
    nc = _CACHE[key]

    res = run_bass_kernel_spmd(nc, in_maps, list(range(NCORES)))
    entity_agg = np.zeros((N_ENT, D), np.float32)
    user_agg = np.zeros((N_USERS, D), np.float32)
    for c in range(NCORES):
        r = res.results[c]
        entity_agg[EPC * c:EPC * (c + 1)] = r["ent_out"][:EPC]
        entity_agg[N_ITEMS + EPC * c:N_ITEMS + EPC * (c + 1)] = r["ent_out"][EPC:]
        user_agg[EPC * c:EPC * (c + 1)] = r["user_out"]
    return user_agg, entity_agg
